# revision 6
# baseline (speedup 1.0000x reference)
"""Trainium2 Bass kernel for nn_DeformableCrossAttention.

Sharding: data-parallel over batch B=8 across 8 NeuronCores (one sample per
core).  Inside each core:
  - offset MLP + attention MLP in fp32 (sampling-position precision matters:
    output error ~ 1.4x the position error in pixels)
  - v = context @ Wv as 3-pass bf16 hi/lo split (hh+hl+lh, ~5e-6 rel err,
    3x faster than native fp32 matmul which runs at 4 cycles/row)
  - bilinear sampling via per-head SWDGE dma_gather of 512B chunks (two
    adjacent-x spatial positions x 64 dh floats) from a per-head
    [spatial, dh] f32 DRAM layout of v
  - attention-weighted bilinear reduce on DVE (elementwise mult with
    broadcast weights + segmented add-reduce)
  - out-projection in fp32, emitted transposed; host transposes back.

Self-contained: hardcodes all shapes from the problem spec.
"""
import sys
sys.path.insert(0, "/opt/trn_rl_repo")

import numpy as np
import concourse.bass as bass
import concourse.mybir as mybir
import concourse.tile as tile
from concourse import bacc
from concourse.bass_utils import run_bass_kernel_spmd
from concourse.masks import make_identity

F32 = mybir.dt.float32
BF16 = mybir.dt.bfloat16
I16 = mybir.dt.int16
I32 = mybir.dt.int32
AF = mybir.ActivationFunctionType
ALU = mybir.AluOpType
AX = mybir.AxisListType

B, N, DIM = 8, 256, 1024
HEADS, DH, P = 16, 64, 8
HS = WS = 64
CTX = HS * WS            # 4096
INNER = HEADS * DH       # 1024
KT = DIM // 128          # 8 k-tiles
PLANE = CTX * DH         # per-head v plane elements (262144)

V_PASSES = 3             # 1: bf16, 3: bf16 hi/lo split (hh + hl + lh)
CTX_SUP = 512            # ctx supertile rows
N_SUP = CTX // CTX_SUP   # 8 supertiles
M_PER_SUP = CTX_SUP // 128

_CACHE = {}


def _ap(t, offset, dims):
    return bass.AP(t.ap().tensor if hasattr(t, "ap") else t.tensor, offset, dims)


def _sap(tile_obj, extra, dims):
    """Sub-AP of an SBUF tile: keep its partition dim, custom free dims,
    extra offset in elements."""
    a = tile_obj[:]
    return bass.AP(a.tensor, a.offset + extra, [list(a.ap[0])] + dims)


def _build():
    nc = bacc.Bacc("TRN2", target_bir_lowering=False, debug=False)

    # ---------------- I/O ----------------
    ctxT_hi = nc.dram_tensor("ctxT_hi", [DIM, CTX], BF16, kind="ExternalInput")
    ctxT_lo = nc.dram_tensor("ctxT_lo", [DIM, CTX], BF16, kind="ExternalInput")
    Wv_hi = nc.dram_tensor("Wv_hi", [DIM, INNER], BF16, kind="ExternalInput")
    Wv_lo = nc.dram_tensor("Wv_lo", [DIM, INNER], BF16, kind="ExternalInput")
    xoffT = nc.dram_tensor("xoffT", [DIM, N], F32, kind="ExternalInput")
    xattT = nc.dram_tensor("xattT", [DIM, N], F32, kind="ExternalInput")
    W_off1 = nc.dram_tensor("W_off1", [DIM, DIM], F32, kind="ExternalInput")
    b_off1 = nc.dram_tensor("b_off1", [DIM], F32, kind="ExternalInput")
    W_off2p = nc.dram_tensor("W_off2p", [DIM, 256], F32, kind="ExternalInput")
    b_off2p = nc.dram_tensor("b_off2p", [256], F32, kind="ExternalInput")
    W_att1 = nc.dram_tensor("W_att1", [DIM, DIM], F32, kind="ExternalInput")
    b_att1 = nc.dram_tensor("b_att1", [DIM], F32, kind="ExternalInput")
    W_att2 = nc.dram_tensor("W_att2", [DIM, 128], F32, kind="ExternalInput")
    b_att2 = nc.dram_tensor("b_att2", [128], F32, kind="ExternalInput")
    W_out = nc.dram_tensor("W_out", [INNER, DIM], F32, kind="ExternalInput")
    b_out = nc.dram_tensor("b_out", [DIM], F32, kind="ExternalInput")

    outT = nc.dram_tensor("outT", [DIM, N], F32, kind="ExternalOutput")

    # DRAM scratch
    v_dram = nc.dram_tensor("v_dram", [HEADS, PLANE], F32)
    # idxC[q, h, cy, p, m] int16
    idxC = nc.dram_tensor("idxC", [16, HEADS, 2, P, 16], I16)

    vwrite_insts = []
    cwrite_insts = []
    gather_insts = []
    idxload_insts = []

    with tile.TileContext(nc) as tc:
        import contextlib
        with contextlib.ExitStack() as ctx:
            persist = ctx.enter_context(tc.tile_pool(name="persist", bufs=1))
            ws = ctx.enter_context(tc.tile_pool(name="wstream", bufs=6))
            h1p = ctx.enter_context(tc.tile_pool(name="h1p", bufs=1))
            ctxp = ctx.enter_context(tc.tile_pool(name="ctxp", bufs=1))
            vsbp = ctx.enter_context(tc.tile_pool(name="vsbp", bufs=2))
            gp = ctx.enter_context(tc.tile_pool(name="gp", bufs=2))
            wtp = ctx.enter_context(tc.tile_pool(name="wtp", bufs=1))
            scr = ctx.enter_context(tc.tile_pool(name="scr", bufs=1))
            mps = ctx.enter_context(tc.tile_pool(name="mps", bufs=2, space="PSUM"))
            vps = ctx.enter_context(tc.tile_pool(name="vps", bufs=2, space="PSUM"))
            tps = ctx.enter_context(tc.tile_pool(name="tps", bufs=2, space="PSUM"))

            # ---------- persistent loads ----------
            def load_tiles(dram, rows, cols, dt, tag, ncols=None):
                ncols = cols if ncols is None else ncols
                ts_ = []
                for k in range(rows // 128):
                    t = persist.tile([128, ncols], dt, tag=f"{tag}_{k}")
                    nc.sync.dma_start(t[:], dram[k * 128:(k + 1) * 128, :])
                    ts_.append(t)
                return ts_

            wv_hi = load_tiles(Wv_hi, DIM, INNER, BF16, "wvh")
            wv_lo = load_tiles(Wv_lo, DIM, INNER, BF16, "wvl") if V_PASSES == 3 else None
            woff2 = load_tiles(W_off2p, DIM, 256, F32, "wo2")
            watt2 = load_tiles(W_att2, DIM, 128, F32, "wa2")

            def load_bias(dram, n_elem, tag):
                k = n_elem // 128
                t = persist.tile([128, k], F32, tag=tag)
                nc.sync.dma_start(t[:], _ap(dram, 0, [[1, 128], [128, k]]))
                return t

            bo1 = load_bias(b_off1, DIM, "bo1")
            bo2 = load_bias(b_off2p, 256, "bo2")
            ba1 = load_bias(b_att1, DIM, "ba1")
            ba2 = load_bias(b_att2, 128, "ba2")
            bo = load_bias(b_out, DIM, "bo")

            ident = persist.tile([128, 128], F32, tag="ident")
            make_identity(nc, ident[:])

            # ---------- MLP helper: yT[m] = act(sum_k W[k,m]^T @ xT[k] + b[m]) ----------
            def mlp_layer(w_dram, x_tiles, bias_tile, mtiles, act, out_tag, pool):
                outs = []
                for m in range(mtiles):
                    ps = mps.tile([128, N], F32, tag="mlp_ps")
                    for k in range(KT):
                        wt = ws.tile([128, 128], F32, tag="wst")
                        nc.sync.dma_start(
                            wt[:], w_dram[k * 128:(k + 1) * 128,
                                          m * 128:(m + 1) * 128])
                        nc.tensor.matmul(ps[:], wt[:], x_tiles[k][:],
                                         start=(k == 0), stop=(k == KT - 1))
                    o = pool.tile([128, N], F32, tag=f"{out_tag}_{m}")
                    nc.scalar.activation(o[:], ps[:], act,
                                         bias=bias_tile[:, m:m + 1])
                    outs.append(o)
                return outs

            def mlp_layer2(w_tiles, x_tiles, bias_tile, mtiles, act, out_tag,
                           use_dve_bias=False):
                outs = []
                for m in range(mtiles):
                    ps = mps.tile([128, N], F32, tag="mlp_ps")
                    for k in range(KT):
                        nc.tensor.matmul(ps[:], w_tiles[k][:, m * 128:(m + 1) * 128],
                                         x_tiles[k][:],
                                         start=(k == 0), stop=(k == KT - 1))
                    o = scr.tile([128, N], F32, tag=f"{out_tag}_{m}")
                    if use_dve_bias:
                        nc.vector.tensor_scalar(o[:], ps[:], bias_tile[:, m:m + 1],
                                                None, op0=ALU.add)
                    else:
                        nc.scalar.activation(o[:], ps[:], act,
                                             bias=bias_tile[:, m:m + 1])
                    outs.append(o)
                return outs

            # ---------- stage A: offset MLP (fp32) ----------
            xoff_t = []
            for k in range(KT):
                t = h1p.tile([128, N], F32, tag=f"xt_{k}")
                nc.sync.dma_start(t[:], xoffT[k * 128:(k + 1) * 128, :])
                xoff_t.append(t)
            h1 = mlp_layer(W_off1, xoff_t, bo1, KT, AF.Gelu, "h1", h1p)
            # off2: 2 m-tiles -> lxT (cy=0), lyT (cy=1), tanh applied
            loff = mlp_layer2(woff2, h1, bo2, 2, AF.Tanh, "loff")
            lxT, lyT = loff

            # ---------- stage B: attention MLP (fp32) ----------
            xatt_t = []
            for k in range(KT):
                t = h1p.tile([128, N], F32, tag=f"xt_{k}")
                nc.sync.dma_start(t[:], xattT[k * 128:(k + 1) * 128, :])
                xatt_t.append(t)
            g1 = mlp_layer(W_att1, xatt_t, ba1, KT, AF.Gelu, "h1", h1p)
            attT = mlp_layer2(watt2, g1, ba2, 1, AF.Copy, "attT",
                              use_dve_bias=True)[0]

            # ---------- stage C: PE transposes to [n, hp] ----------
            def transpose_128x256(src, tag):
                halves = []
                for i in range(2):
                    pt = tps.tile([128, 128], F32, tag="trps")
                    nc.tensor.transpose(pt[:], src[:, i * 128:(i + 1) * 128],
                                        ident[:])
                    o = scr.tile([128, 128], F32, tag=f"{tag}_{i}")
                    nc.vector.tensor_copy(o[:], pt[:])
                    halves.append(o)
                return halves

            lx_n = transpose_128x256(lxT, "lxn")   # [n-tile][128, 128hp]
            ly_n = transpose_128x256(lyT, "lyn")
            att_n = transpose_128x256(attT, "attn")

            # ---------- stage D1: positions in [hp, n] for gather indices ----------
            # g' = tanh*31.5 + 31.0 ; min 62.4999 ; xi = round(g'), xf float
            def pos_chain_T(lt, tag):
                gp_ = scr.tile([128, N], F32, tag=f"{tag}_g")
                nc.vector.tensor_scalar(gp_[:], lt[:], 31.5, 31.0,
                                        op0=ALU.mult, op1=ALU.add)
                nc.vector.tensor_scalar(gp_[:], gp_[:], 62.4999, None, op0=ALU.min)
                xi = scr.tile([128, N], I32, tag=f"{tag}_i")
                nc.vector.tensor_copy(xi[:], gp_[:])
                xf = scr.tile([128, N], F32, tag=f"{tag}_f")
                nc.vector.tensor_copy(xf[:], xi[:])
                return gp_, xf

            _, xfT = pos_chain_T(lxT, "pxT")
            _, yfT = pos_chain_T(lyT, "pyT")
            idx0f = scr.tile([128, N], F32, tag="idx0f")
            nc.vector.scalar_tensor_tensor(idx0f[:], yfT[:], 64.0, xfT[:],
                                           op0=ALU.mult, op1=ALU.add)
            idx1f = scr.tile([128, N], F32, tag="idx1f")
            nc.vector.tensor_scalar(idx1f[:], idx0f[:], 64.0, None, op0=ALU.add)

            for cy, srcf in enumerate([idx0f, idx1f]):
                ii = scr.tile([128, N], I16, tag=f"idxi_{cy}")
                nc.vector.tensor_copy(ii[:], srcf[:])
                # free transpose: Sg[hp, q*16+m] = ii[hp, m*16+q]
                sg = scr.tile([128, N], I16, tag=f"sg_{cy}")
                nc.vector.tensor_copy(
                    sg[:], _sap(ii, 0, [[1, 16], [16, 16]]))
                # write to idxC[q, h, cy, p, m]: one DMA per q (3-dim AP cap)
                for q in range(16):
                    dst = bass.AP(idxC.ap().tensor, q * 4096 + cy * 128,
                                  [[256, 16], [16, 8], [1, 16]])
                    w = nc.sync.dma_start(dst, sg[:, q * 16:(q + 1) * 16])
                    cwrite_insts.append(w)

            # ---------- stage D2: lerp weights + softmax in [n, hp] ----------
            # t[n, col = h*32 + cy*16 + p*2 + half]
            t_tiles = []
            for nh in range(2):
                gx_ = scr.tile([128, 128], F32, tag=f"gxn_{nh}")
                nc.vector.tensor_scalar(gx_[:], lx_n[nh][:], 31.5, 31.0,
                                        op0=ALU.mult, op1=ALU.add)
                nc.vector.tensor_scalar(gx_[:], gx_[:], 62.4999, None, op0=ALU.min)
                xi_ = scr.tile([128, 128], I32, tag=f"xin_{nh}")
                nc.vector.tensor_copy(xi_[:], gx_[:])
                xf_ = scr.tile([128, 128], F32, tag=f"xfn_{nh}")
                nc.vector.tensor_copy(xf_[:], xi_[:])
                wx = scr.tile([128, 128], F32, tag=f"wx_{nh}")
                # wx = (g' + 0.5) - xf
                nc.vector.scalar_tensor_tensor(wx[:], gx_[:], 0.5, xf_[:],
                                               op0=ALU.add, op1=ALU.subtract)
                gy_ = scr.tile([128, 128], F32, tag=f"gyn_{nh}")
                nc.vector.tensor_scalar(gy_[:], ly_n[nh][:], 31.5, 31.0,
                                        op0=ALU.mult, op1=ALU.add)
                nc.vector.tensor_scalar(gy_[:], gy_[:], 62.4999, None, op0=ALU.min)
                yi_ = scr.tile([128, 128], I32, tag=f"yin_{nh}")
                nc.vector.tensor_copy(yi_[:], gy_[:])
                yf_ = scr.tile([128, 128], F32, tag=f"yfn_{nh}")
                nc.vector.tensor_copy(yf_[:], yi_[:])
                wy = scr.tile([128, 128], F32, tag=f"wy_{nh}")
                nc.vector.scalar_tensor_tensor(wy[:], gy_[:], 0.5, yf_[:],
                                               op0=ALU.add, op1=ALU.subtract)

                # softmax over p (groups of 8 along free)
                an = att_n[nh]
                mx = scr.tile([128, 16], F32, tag=f"mx_{nh}")
                nc.vector.tensor_reduce(
                    mx[:], _sap(an, 0, [[8, 16], [1, 8]]),
                    axis=AX.X, op=ALU.max)
                ex = scr.tile([128, 128], F32, tag=f"ex_{nh}")
                nc.vector.tensor_tensor(
                    _sap(ex, 0, [[8, 16], [1, 8]]),
                    _sap(an, 0, [[8, 16], [1, 8]]),
                    _sap(mx, 0, [[1, 16], [0, 8]]),
                    op=ALU.subtract)
                nc.scalar.activation(ex[:], ex[:], AF.Exp)
                sm = scr.tile([128, 16], F32, tag=f"sm_{nh}")
                nc.vector.tensor_reduce(
                    sm[:], _sap(ex, 0, [[8, 16], [1, 8]]),
                    axis=AX.X, op=ALU.add)
                rs = scr.tile([128, 16], F32, tag=f"rs_{nh}")
                nc.vector.reciprocal(rs[:], sm[:])
                aw = scr.tile([128, 128], F32, tag=f"aw_{nh}")
                nc.vector.tensor_tensor(
                    _sap(aw, 0, [[8, 16], [1, 8]]),
                    _sap(ex, 0, [[8, 16], [1, 8]]),
                    _sap(rs, 0, [[1, 16], [0, 8]]),
                    op=ALU.mult)

                # u0 = aw*(1-wx) = aw - aw*wx ; u1 = aw*wx
                u1 = scr.tile([128, 128], F32, tag=f"u1_{nh}")
                nc.vector.tensor_tensor(u1[:], aw[:], wx[:], op=ALU.mult)
                u0 = scr.tile([128, 128], F32, tag=f"u0_{nh}")
                nc.vector.tensor_tensor(u0[:], aw[:], u1[:], op=ALU.subtract)
                cw1 = wy
                cw0 = scr.tile([128, 128], F32, tag=f"cw0_{nh}")
                nc.vector.tensor_scalar(cw0[:], wy[:], -1.0, 1.0,
                                        op0=ALU.mult, op1=ALU.add)

                tt = scr.tile([128, 512], F32, tag=f"tt_{nh}")
                for cyv, cw in ((0, cw0), (1, cw1)):
                    for half, u in ((0, u0), (1, u1)):
                        nc.vector.tensor_tensor(
                            _sap(tt, cyv * 16 + half, [[32, 16], [2, 8]]),
                            _sap(u, 0, [[8, 16], [1, 8]]),
                            _sap(cw, 0, [[8, 16], [1, 8]]),
                            op=ALU.mult)
                t_tiles.append(tt)

            # ---------- stage E: v matmul + store ----------
            for sup in range(N_SUP):
                c0 = sup * CTX_SUP
                chi, clo = [], []
                for k in range(KT):
                    t = ctxp.tile([128, CTX_SUP], BF16, tag=f"ch_{k}")
                    nc.sync.dma_start(t[:], ctxT_hi[k * 128:(k + 1) * 128,
                                                    c0:c0 + CTX_SUP])
                    chi.append(t)
                    if V_PASSES == 3:
                        t2 = ctxp.tile([128, CTX_SUP], BF16, tag=f"cl_{k}")
                        nc.sync.dma_start(t2[:], ctxT_lo[k * 128:(k + 1) * 128,
                                                         c0:c0 + CTX_SUP])
                        clo.append(t2)
                for mm in range(M_PER_SUP):
                    msl = slice(mm * 128, (mm + 1) * 128)
                    pss = [vps.tile([128, 512], F32, tag=f"vps_{h2}",
                                    name=f"vps_{sup}_{mm}_{h2}")
                           for h2 in range(2)]
                    passes = [(chi, wv_hi)]
                    if V_PASSES == 3:
                        passes += [(chi, wv_lo), (clo, wv_hi)]
                    np_ = len(passes)
                    for pi, (lhs_t, rhs_t) in enumerate(passes):
                        for k in range(KT):
                            first = (pi == 0 and k == 0)
                            last = (pi == np_ - 1 and k == KT - 1)
                            for h2 in range(2):
                                nc.tensor.matmul(
                                    pss[h2][:], lhs_t[k][:, msl],
                                    rhs_t[k][:, h2 * 512:(h2 + 1) * 512],
                                    start=first, stop=last)
                    vsb = vsbp.tile([128, 1024], F32, tag="vsb")
                    for h2 in range(2):
                        nc.scalar.copy(vsb[:, h2 * 512:(h2 + 1) * 512], pss[h2][:])
                    # store: v_dram[h, (c0+mm*128 + r)*64 + d]
                    dst = bass.AP(v_dram.ap().tensor, (c0 + mm * 128) * DH,
                                  [[DH, 128], [PLANE, HEADS], [1, DH]])
                    w = nc.sync.dma_start(dst, vsb[:])
                    vwrite_insts.append(w)

            # ---------- stage F: per-head gather + reduce ----------
            z = persist.tile([128, 2048], F32, tag="z")  # col = nh*1024 + h*64 + d
            # all-heads idx tile [128, 4096]: col = h*256 + cy*128 + p*16 + m,
            # partitions = 8 replicas x 16 q
            all_idx = persist.tile([128, 4096], I16, tag="all_idx")
            for gi_ in range(8):
                src = bass.AP(idxC.ap().tensor, 0, [[4096, 16], [1, 4096]])
                ld = nc.sync.dma_start(all_idx[gi_ * 16:(gi_ + 1) * 16, :], src)
                idxload_insts.append(ld)

            for h in range(HEADS):
                g = gp.tile([128, 32, 128], F32, tag="g")
                vsrc = bass.AP(v_dram.ap().tensor, h * PLANE,
                               [[DH, CTX - 1], [1, 2 * DH]])
                gi = nc.gpsimd.dma_gather(g[:], vsrc,
                                          all_idx[:, h * 256:(h + 1) * 256],
                                          4096, 4096,
                                          2 * DH, elem_step=DH,
                                          single_packet=False)
                gather_insts.append(gi)

                for nh in range(2):
                    wt = wtp.tile([128, 2048], F32, tag="wt")
                    # wt[d*32 + s*2 + half] = G[n, chunk s*2+nh, half*64+d] * t
                    nc.vector.tensor_tensor(
                        _sap(wt, 0, [[32, 64], [2, 16], [1, 2]]),
                        _sap(g, nh * 128, [[1, 64], [256, 16], [64, 2]]),
                        _sap(t_tiles[nh], h * 32, [[0, 64], [2, 16], [1, 2]]),
                        op=ALU.mult)
                    nc.vector.tensor_reduce(
                        _sap(z, nh * 1024 + h * 64, [[1, 64]]),
                        _sap(wt, 0, [[32, 64], [1, 32]]),
                        axis=AX.X, op=ALU.add)

            # ---------- stage G: z -> zT, out projection ----------
            zT = []
            for k in range(KT):
                t = persist.tile([128, N], F32, tag=f"zT_{k}")
                zT.append(t)
            for nh in range(2):
                for c in range(8):
                    pt = tps.tile([128, 128], F32, tag="trps")
                    nc.tensor.transpose(
                        pt[:], z[:, nh * 1024 + c * 128: nh * 1024 + (c + 1) * 128],
                        ident[:])
                    nc.vector.tensor_copy(zT[c][:, nh * 128:(nh + 1) * 128], pt[:])

            for m in range(KT):
                ps = mps.tile([128, N], F32, tag="mlp_ps")
                for k in range(KT):
                    wt = ws.tile([128, 128], F32, tag="wst")
                    nc.sync.dma_start(
                        wt[:], W_out[k * 128:(k + 1) * 128, m * 128:(m + 1) * 128])
                    nc.tensor.matmul(ps[:], wt[:], zT[k][:],
                                     start=(k == 0), stop=(k == KT - 1))
                osb = scr.tile([128, N], F32, tag="osb")
                nc.vector.tensor_scalar(osb[:], ps[:], bo[:, m:m + 1], None,
                                        op0=ALU.add)
                nc.sync.dma_start(outT[m * 128:(m + 1) * 128, :], osb[:])

            # ---------- explicit DRAM deps (gather after v/idx writes) ----------
            from concourse.tile_rust import add_dep_helper
            for gi in gather_insts:
                for w in vwrite_insts:
                    add_dep_helper(gi.ins, w.ins, reason="gather after v write")
            for ld in idxload_insts:
                for w in cwrite_insts:
                    add_dep_helper(ld.ins, w.ins, reason="idx load after idxC write")

    nc.compile()
    return nc


def _prep_inputs(inputs):
    x = np.ascontiguousarray(np.asarray(inputs["x"], dtype=np.float32))
    context = np.asarray(inputs["context"], dtype=np.float32)
    Wv = np.asarray(inputs["Wv"], dtype=np.float32)
    W_off2 = np.asarray(inputs["W_off2"], dtype=np.float32)
    b_off2 = np.asarray(inputs["b_off2"], dtype=np.float32)

    def split(a):
        hi = a.astype(np.bfloat16) if hasattr(np, "bfloat16") else None
        import ml_dtypes
        hi = a.astype(ml_dtypes.bfloat16)
        lo = (a - hi.astype(np.float32)).astype(ml_dtypes.bfloat16)
        return hi, lo

    Wv_hi, Wv_lo = split(Wv)

    cy, h, p = np.meshgrid(np.arange(2), np.arange(HEADS), np.arange(P),
                           indexing="ij")
    oldcol = (h * 16 + p * 2 + cy).reshape(-1)
    W_off2p = np.ascontiguousarray(W_off2[:, oldcol])
    b_off2p = np.ascontiguousarray(b_off2[oldcol])

    common = {
        "Wv_hi": Wv_hi, "Wv_lo": Wv_lo,
        "W_off1": np.ascontiguousarray(np.asarray(inputs["W_off1"], np.float32)),
        "b_off1": np.asarray(inputs["b_off1"], np.float32),
        "W_off2p": W_off2p, "b_off2p": b_off2p,
        "W_att1": np.ascontiguousarray(np.asarray(inputs["W_att1"], np.float32)),
        "b_att1": np.asarray(inputs["b_att1"], np.float32),
        "W_att2": np.ascontiguousarray(np.asarray(inputs["W_att2"], np.float32)),
        "b_att2": np.asarray(inputs["b_att2"], np.float32),
        "W_out": np.ascontiguousarray(np.asarray(inputs["W_out"], np.float32)),
        "b_out": np.asarray(inputs["b_out"], np.float32),
    }

    xflat = x.reshape(B * N, DIM)
    n_idx = np.arange(N)
    in_maps = []
    for b in range(B):
        perm = (n_idx // 32) * 256 + (n_idx % 32) * 8 + b
        xoffT = np.ascontiguousarray(xflat[perm].T)
        xattT = np.ascontiguousarray(x[b].T)
        ctxT = np.ascontiguousarray(context[b].T)
        c_hi, c_lo = split(ctxT)
        m = dict(common)
        m.update({"ctxT_hi": c_hi, "ctxT_lo": c_lo,
                  "xoffT": xoffT, "xattT": xattT})
        in_maps.append(m)
    return in_maps


def kernel(**inputs):
    if "nc" not in _CACHE:
        _CACHE["nc"] = _build()
    nc = _CACHE["nc"]
    in_maps = _prep_inputs(inputs)
    res = run_bass_kernel_spmd(nc, in_maps, list(range(8)))
    _CACHE["last_results"] = res
    out = np.stack([res.results[i]["outT"].T for i in range(B)], axis=0)
    return np.ascontiguousarray(out.astype(np.float32))


# revision 7
# speedup vs baseline: 83.2229x; 83.2229x over previous
"""Trainium2 Bass kernel for nn_DeformableCrossAttention.

Sharding: data-parallel over batch B=8 across 8 NeuronCores (one sample per
core).  Inside each core:
  - offset MLP + attention MLP in fp32 (sampling-position precision matters:
    output error ~ 1.4x the position error in pixels)
  - v = context @ Wv as 3-pass bf16 hi/lo split (hh+hl+lh, ~5e-6 rel err,
    3x faster than native fp32 matmul which runs at 4 cycles/row)
  - bilinear sampling via per-head SWDGE dma_gather of 512B chunks (two
    adjacent-x spatial positions x 64 dh floats) from a per-head
    [spatial, dh] f32 DRAM layout of v
  - attention-weighted bilinear reduce on DVE (elementwise mult with
    broadcast weights + segmented add-reduce)
  - out-projection in fp32, emitted transposed; host transposes back.

Self-contained: hardcodes all shapes from the problem spec.
"""
import sys
sys.path.insert(0, "/opt/trn_rl_repo")

import numpy as np
import concourse.bass as bass
import concourse.mybir as mybir
import concourse.tile as tile
from concourse import bacc
from concourse.bass_utils import run_bass_kernel_spmd
from concourse.masks import make_identity

F32 = mybir.dt.float32
BF16 = mybir.dt.bfloat16
I16 = mybir.dt.int16
I32 = mybir.dt.int32
AF = mybir.ActivationFunctionType
ALU = mybir.AluOpType
AX = mybir.AxisListType

B, N, DIM = 8, 256, 1024
HEADS, DH, P = 16, 64, 8
HS = WS = 64
CTX = HS * WS            # 4096
INNER = HEADS * DH       # 1024
KT = DIM // 128          # 8 k-tiles
PLANE = CTX * DH         # per-head v plane elements (262144)

V_PASSES = 3             # 1: bf16, 3: bf16 hi/lo split (hh + hl + lh)
CTX_SUP = 512            # ctx supertile rows
N_SUP = CTX // CTX_SUP   # 8 supertiles
M_PER_SUP = CTX_SUP // 128

_CACHE = {}


def _ap(t, offset, dims):
    return bass.AP(t.ap().tensor if hasattr(t, "ap") else t.tensor, offset, dims)


def _sap(tile_obj, extra, dims):
    """Sub-AP of an SBUF tile: keep its partition dim, custom free dims,
    extra offset in elements."""
    a = tile_obj[:]
    return bass.AP(a.tensor, a.offset + extra, [list(a.ap[0])] + dims)


def _build(repeat=1):
    nc = bacc.Bacc("TRN2", target_bir_lowering=False, debug=False)

    # ---------------- I/O ----------------
    ctxT_hi = nc.dram_tensor("ctxT_hi", [DIM, CTX], BF16, kind="ExternalInput")
    ctxT_lo = nc.dram_tensor("ctxT_lo", [DIM, CTX], BF16, kind="ExternalInput")
    Wv_hi = nc.dram_tensor("Wv_hi", [DIM, INNER], BF16, kind="ExternalInput")
    Wv_lo = nc.dram_tensor("Wv_lo", [DIM, INNER], BF16, kind="ExternalInput")
    xoffT = nc.dram_tensor("xoffT", [DIM, N], F32, kind="ExternalInput")
    xattT = nc.dram_tensor("xattT", [DIM, N], F32, kind="ExternalInput")
    W_off1 = nc.dram_tensor("W_off1", [DIM, DIM], F32, kind="ExternalInput")
    b_off1 = nc.dram_tensor("b_off1", [DIM], F32, kind="ExternalInput")
    W_off2p = nc.dram_tensor("W_off2p", [DIM, 256], F32, kind="ExternalInput")
    b_off2p = nc.dram_tensor("b_off2p", [256], F32, kind="ExternalInput")
    W_att1 = nc.dram_tensor("W_att1", [DIM, DIM], F32, kind="ExternalInput")
    b_att1 = nc.dram_tensor("b_att1", [DIM], F32, kind="ExternalInput")
    W_att2 = nc.dram_tensor("W_att2", [DIM, 128], F32, kind="ExternalInput")
    b_att2 = nc.dram_tensor("b_att2", [128], F32, kind="ExternalInput")
    W_out = nc.dram_tensor("W_out", [INNER, DIM], F32, kind="ExternalInput")
    b_out = nc.dram_tensor("b_out", [DIM], F32, kind="ExternalInput")

    outT = nc.dram_tensor("outT", [DIM, N], F32, kind="ExternalOutput")

    # DRAM scratch
    v_dram = nc.dram_tensor("v_dram", [HEADS, PLANE], F32)
    # idxC[q, h, cy, p, m] int16
    idxC = nc.dram_tensor("idxC", [16, HEADS, 2, P, 16], I16)

    vwrite_insts = []
    cwrite_insts = []
    gather_insts = []
    idxload_insts = []

    with tile.TileContext(nc) as tc:
        import contextlib
        with contextlib.ExitStack() as ctx:
            persist = ctx.enter_context(tc.tile_pool(name="persist", bufs=1))
            ws = ctx.enter_context(tc.tile_pool(name="wstream", bufs=6))
            h1p = ctx.enter_context(tc.tile_pool(name="h1p", bufs=1))
            ctxp = ctx.enter_context(tc.tile_pool(name="ctxp", bufs=1))
            vsbp = ctx.enter_context(tc.tile_pool(name="vsbp", bufs=2))
            gp = ctx.enter_context(tc.tile_pool(name="gp", bufs=2))
            wtp = ctx.enter_context(tc.tile_pool(name="wtp", bufs=1))
            scr = ctx.enter_context(tc.tile_pool(name="scr", bufs=1))
            mps = ctx.enter_context(tc.tile_pool(name="mps", bufs=2, space="PSUM"))
            vps = ctx.enter_context(tc.tile_pool(name="vps", bufs=2, space="PSUM"))
            tps = ctx.enter_context(tc.tile_pool(name="tps", bufs=2, space="PSUM"))
            if repeat > 1:
                ctx.enter_context(tc.For_i(0, repeat, 1))

            # ---------- persistent loads ----------
            def load_tiles(dram, rows, cols, dt, tag, ncols=None):
                ncols = cols if ncols is None else ncols
                ts_ = []
                for k in range(rows // 128):
                    t = persist.tile([128, ncols], dt, tag=f"{tag}_{k}")
                    nc.sync.dma_start(t[:], dram[k * 128:(k + 1) * 128, :])
                    ts_.append(t)
                return ts_

            wv_hi = load_tiles(Wv_hi, DIM, INNER, BF16, "wvh")
            wv_lo = load_tiles(Wv_lo, DIM, INNER, BF16, "wvl") if V_PASSES == 3 else None
            woff2 = load_tiles(W_off2p, DIM, 256, F32, "wo2")
            watt2 = load_tiles(W_att2, DIM, 128, F32, "wa2")

            def load_bias(dram, n_elem, tag):
                k = n_elem // 128
                t = persist.tile([128, k], F32, tag=tag)
                nc.sync.dma_start(t[:], _ap(dram, 0, [[1, 128], [128, k]]))
                return t

            bo1 = load_bias(b_off1, DIM, "bo1")
            bo2 = load_bias(b_off2p, 256, "bo2")
            ba1 = load_bias(b_att1, DIM, "ba1")
            ba2 = load_bias(b_att2, 128, "ba2")
            bo = load_bias(b_out, DIM, "bo")

            ident = persist.tile([128, 128], F32, tag="ident")
            make_identity(nc, ident[:])

            # ---------- MLP helper: yT[m] = act(sum_k W[k,m]^T @ xT[k] + b[m]) ----------
            def mlp_layer(w_dram, x_tiles, bias_tile, mtiles, act, out_tag, pool):
                outs = []
                for m in range(mtiles):
                    ps = mps.tile([128, N], F32, tag="mlp_ps")
                    for k in range(KT):
                        wt = ws.tile([128, 128], F32, tag="wst")
                        nc.sync.dma_start(
                            wt[:], w_dram[k * 128:(k + 1) * 128,
                                          m * 128:(m + 1) * 128])
                        nc.tensor.matmul(ps[:], wt[:], x_tiles[k][:],
                                         start=(k == 0), stop=(k == KT - 1))
                    o = pool.tile([128, N], F32, tag=f"{out_tag}_{m}")
                    nc.scalar.activation(o[:], ps[:], act,
                                         bias=bias_tile[:, m:m + 1])
                    outs.append(o)
                return outs

            def mlp_layer2(w_tiles, x_tiles, bias_tile, mtiles, act, out_tag,
                           use_dve_bias=False):
                outs = []
                for m in range(mtiles):
                    ps = mps.tile([128, N], F32, tag="mlp_ps")
                    for k in range(KT):
                        nc.tensor.matmul(ps[:], w_tiles[k][:, m * 128:(m + 1) * 128],
                                         x_tiles[k][:],
                                         start=(k == 0), stop=(k == KT - 1))
                    o = scr.tile([128, N], F32, tag=f"{out_tag}_{m}")
                    if use_dve_bias:
                        nc.vector.tensor_scalar(o[:], ps[:], bias_tile[:, m:m + 1],
                                                None, op0=ALU.add)
                    else:
                        nc.scalar.activation(o[:], ps[:], act,
                                             bias=bias_tile[:, m:m + 1])
                    outs.append(o)
                return outs

            # ---------- stage A: offset MLP (fp32) ----------
            xoff_t = []
            for k in range(KT):
                t = h1p.tile([128, N], F32, tag=f"xt_{k}")
                nc.sync.dma_start(t[:], xoffT[k * 128:(k + 1) * 128, :])
                xoff_t.append(t)
            h1 = mlp_layer(W_off1, xoff_t, bo1, KT, AF.Gelu, "h1", h1p)
            # off2: 2 m-tiles -> lxT (cy=0), lyT (cy=1), tanh applied
            loff = mlp_layer2(woff2, h1, bo2, 2, AF.Tanh, "loff")
            lxT, lyT = loff

            # ---------- stage B: attention MLP (fp32) ----------
            xatt_t = []
            for k in range(KT):
                t = h1p.tile([128, N], F32, tag=f"xt_{k}")
                nc.sync.dma_start(t[:], xattT[k * 128:(k + 1) * 128, :])
                xatt_t.append(t)
            g1 = mlp_layer(W_att1, xatt_t, ba1, KT, AF.Gelu, "h1", h1p)
            attT = mlp_layer2(watt2, g1, ba2, 1, AF.Copy, "attT",
                              use_dve_bias=True)[0]

            # ---------- stage C: PE transposes to [n, hp] ----------
            def transpose_128x256(src, tag):
                halves = []
                for i in range(2):
                    pt = tps.tile([128, 128], F32, tag="trps")
                    nc.tensor.transpose(pt[:], src[:, i * 128:(i + 1) * 128],
                                        ident[:])
                    o = scr.tile([128, 128], F32, tag=f"{tag}_{i}")
                    nc.vector.tensor_copy(o[:], pt[:])
                    halves.append(o)
                return halves

            lx_n = transpose_128x256(lxT, "lxn")   # [n-tile][128, 128hp]
            ly_n = transpose_128x256(lyT, "lyn")
            att_n = transpose_128x256(attT, "attn")

            # ---------- stage D1: positions in [hp, n] for gather indices ----------
            # g' = tanh*31.5 + 31.0 ; min 62.4999 ; xi = round(g'), xf float
            def pos_chain_T(lt, tag):
                gp_ = scr.tile([128, N], F32, tag=f"{tag}_g")
                nc.vector.tensor_scalar(gp_[:], lt[:], 31.5, 31.0,
                                        op0=ALU.mult, op1=ALU.add)
                nc.vector.tensor_scalar(gp_[:], gp_[:], 62.4999, None, op0=ALU.min)
                xi = scr.tile([128, N], I32, tag=f"{tag}_i")
                nc.vector.tensor_copy(xi[:], gp_[:])
                xf = scr.tile([128, N], F32, tag=f"{tag}_f")
                nc.vector.tensor_copy(xf[:], xi[:])
                return gp_, xf

            _, xfT = pos_chain_T(lxT, "pxT")
            _, yfT = pos_chain_T(lyT, "pyT")
            idx0f = scr.tile([128, N], F32, tag="idx0f")
            nc.vector.scalar_tensor_tensor(idx0f[:], yfT[:], 64.0, xfT[:],
                                           op0=ALU.mult, op1=ALU.add)
            idx1f = scr.tile([128, N], F32, tag="idx1f")
            nc.vector.tensor_scalar(idx1f[:], idx0f[:], 64.0, None, op0=ALU.add)

            for cy, srcf in enumerate([idx0f, idx1f]):
                ii = scr.tile([128, N], I16, tag=f"idxi_{cy}")
                nc.vector.tensor_copy(ii[:], srcf[:])
                # free transpose: Sg[hp, q*16+m] = ii[hp, m*16+q]
                sg = scr.tile([128, N], I16, tag=f"sg_{cy}")
                nc.vector.tensor_copy(
                    sg[:], _sap(ii, 0, [[1, 16], [16, 16]]))
                # write to idxC[q, h, cy, p, m]: one DMA per q (3-dim AP cap)
                for q in range(16):
                    dst = bass.AP(idxC.ap().tensor, q * 4096 + cy * 128,
                                  [[256, 16], [16, 8], [1, 16]])
                    w = nc.sync.dma_start(dst, sg[:, q * 16:(q + 1) * 16])
                    cwrite_insts.append(w)

            # ---------- stage D2: lerp weights + softmax in [n, hp] ----------
            # t[n, col = h*32 + cy*16 + p*2 + half]
            t_tiles = []
            for nh in range(2):
                gx_ = scr.tile([128, 128], F32, tag=f"gxn_{nh}")
                nc.vector.tensor_scalar(gx_[:], lx_n[nh][:], 31.5, 31.0,
                                        op0=ALU.mult, op1=ALU.add)
                nc.vector.tensor_scalar(gx_[:], gx_[:], 62.4999, None, op0=ALU.min)
                xi_ = scr.tile([128, 128], I32, tag=f"xin_{nh}")
                nc.vector.tensor_copy(xi_[:], gx_[:])
                xf_ = scr.tile([128, 128], F32, tag=f"xfn_{nh}")
                nc.vector.tensor_copy(xf_[:], xi_[:])
                wx = scr.tile([128, 128], F32, tag=f"wx_{nh}")
                # wx = (g' + 0.5) - xf
                nc.vector.scalar_tensor_tensor(wx[:], gx_[:], 0.5, xf_[:],
                                               op0=ALU.add, op1=ALU.subtract)
                gy_ = scr.tile([128, 128], F32, tag=f"gyn_{nh}")
                nc.vector.tensor_scalar(gy_[:], ly_n[nh][:], 31.5, 31.0,
                                        op0=ALU.mult, op1=ALU.add)
                nc.vector.tensor_scalar(gy_[:], gy_[:], 62.4999, None, op0=ALU.min)
                yi_ = scr.tile([128, 128], I32, tag=f"yin_{nh}")
                nc.vector.tensor_copy(yi_[:], gy_[:])
                yf_ = scr.tile([128, 128], F32, tag=f"yfn_{nh}")
                nc.vector.tensor_copy(yf_[:], yi_[:])
                wy = scr.tile([128, 128], F32, tag=f"wy_{nh}")
                nc.vector.scalar_tensor_tensor(wy[:], gy_[:], 0.5, yf_[:],
                                               op0=ALU.add, op1=ALU.subtract)

                # softmax over p (groups of 8 along free)
                an = att_n[nh]
                mx = scr.tile([128, 16], F32, tag=f"mx_{nh}")
                nc.vector.tensor_reduce(
                    mx[:], _sap(an, 0, [[8, 16], [1, 8]]),
                    axis=AX.X, op=ALU.max)
                ex = scr.tile([128, 128], F32, tag=f"ex_{nh}")
                nc.vector.tensor_tensor(
                    _sap(ex, 0, [[8, 16], [1, 8]]),
                    _sap(an, 0, [[8, 16], [1, 8]]),
                    _sap(mx, 0, [[1, 16], [0, 8]]),
                    op=ALU.subtract)
                nc.scalar.activation(ex[:], ex[:], AF.Exp)
                sm = scr.tile([128, 16], F32, tag=f"sm_{nh}")
                nc.vector.tensor_reduce(
                    sm[:], _sap(ex, 0, [[8, 16], [1, 8]]),
                    axis=AX.X, op=ALU.add)
                rs = scr.tile([128, 16], F32, tag=f"rs_{nh}")
                nc.vector.reciprocal(rs[:], sm[:])
                aw = scr.tile([128, 128], F32, tag=f"aw_{nh}")
                nc.vector.tensor_tensor(
                    _sap(aw, 0, [[8, 16], [1, 8]]),
                    _sap(ex, 0, [[8, 16], [1, 8]]),
                    _sap(rs, 0, [[1, 16], [0, 8]]),
                    op=ALU.mult)

                # u0 = aw*(1-wx) = aw - aw*wx ; u1 = aw*wx
                u1 = scr.tile([128, 128], F32, tag=f"u1_{nh}")
                nc.vector.tensor_tensor(u1[:], aw[:], wx[:], op=ALU.mult)
                u0 = scr.tile([128, 128], F32, tag=f"u0_{nh}")
                nc.vector.tensor_tensor(u0[:], aw[:], u1[:], op=ALU.subtract)
                cw1 = wy
                cw0 = scr.tile([128, 128], F32, tag=f"cw0_{nh}")
                nc.vector.tensor_scalar(cw0[:], wy[:], -1.0, 1.0,
                                        op0=ALU.mult, op1=ALU.add)

                tt = scr.tile([128, 512], F32, tag=f"tt_{nh}")
                for cyv, cw in ((0, cw0), (1, cw1)):
                    for half, u in ((0, u0), (1, u1)):
                        nc.vector.tensor_tensor(
                            _sap(tt, cyv * 16 + half, [[32, 16], [2, 8]]),
                            _sap(u, 0, [[8, 16], [1, 8]]),
                            _sap(cw, 0, [[8, 16], [1, 8]]),
                            op=ALU.mult)
                t_tiles.append(tt)

            # ---------- stage E: v matmul + store ----------
            for sup in range(N_SUP):
                c0 = sup * CTX_SUP
                chi, clo = [], []
                for k in range(KT):
                    t = ctxp.tile([128, CTX_SUP], BF16, tag=f"ch_{k}")
                    nc.sync.dma_start(t[:], ctxT_hi[k * 128:(k + 1) * 128,
                                                    c0:c0 + CTX_SUP])
                    chi.append(t)
                    if V_PASSES == 3:
                        t2 = ctxp.tile([128, CTX_SUP], BF16, tag=f"cl_{k}")
                        nc.sync.dma_start(t2[:], ctxT_lo[k * 128:(k + 1) * 128,
                                                         c0:c0 + CTX_SUP])
                        clo.append(t2)
                for mm in range(M_PER_SUP):
                    msl = slice(mm * 128, (mm + 1) * 128)
                    pss = [vps.tile([128, 512], F32, tag=f"vps_{h2}",
                                    name=f"vps_{sup}_{mm}_{h2}")
                           for h2 in range(2)]
                    passes = [(chi, wv_hi)]
                    if V_PASSES == 3:
                        passes += [(chi, wv_lo), (clo, wv_hi)]
                    np_ = len(passes)
                    for pi, (lhs_t, rhs_t) in enumerate(passes):
                        for k in range(KT):
                            first = (pi == 0 and k == 0)
                            last = (pi == np_ - 1 and k == KT - 1)
                            for h2 in range(2):
                                nc.tensor.matmul(
                                    pss[h2][:], lhs_t[k][:, msl],
                                    rhs_t[k][:, h2 * 512:(h2 + 1) * 512],
                                    start=first, stop=last)
                    vsb = vsbp.tile([128, 1024], F32, tag="vsb")
                    for h2 in range(2):
                        nc.scalar.copy(vsb[:, h2 * 512:(h2 + 1) * 512], pss[h2][:])
                    # store: v_dram[h, (c0+mm*128 + r)*64 + d]
                    dst = bass.AP(v_dram.ap().tensor, (c0 + mm * 128) * DH,
                                  [[DH, 128], [PLANE, HEADS], [1, DH]])
                    w = nc.sync.dma_start(dst, vsb[:])
                    vwrite_insts.append(w)

            # ---------- stage F: per-head gather + reduce ----------
            z = persist.tile([128, 2048], F32, tag="z")  # col = nh*1024 + h*64 + d
            # all-heads idx tile [128, 4096]: col = h*256 + cy*128 + p*16 + m,
            # partitions = 8 replicas x 16 q
            all_idx = persist.tile([128, 4096], I16, tag="all_idx")
            for gi_ in range(8):
                src = bass.AP(idxC.ap().tensor, 0, [[4096, 16], [1, 4096]])
                ld = nc.sync.dma_start(all_idx[gi_ * 16:(gi_ + 1) * 16, :], src)
                idxload_insts.append(ld)

            for h in range(HEADS):
                g = gp.tile([128, 32, 128], F32, tag="g")
                vsrc = bass.AP(v_dram.ap().tensor, h * PLANE,
                               [[DH, CTX - 1], [1, 2 * DH]])
                gi = nc.gpsimd.dma_gather(g[:], vsrc,
                                          all_idx[:, h * 256:(h + 1) * 256],
                                          4096, 4096,
                                          2 * DH, elem_step=DH,
                                          single_packet=False)
                gather_insts.append(gi)

                for nh in range(2):
                    wt = wtp.tile([128, 2048], F32, tag="wt")
                    # wt[d*32 + s*2 + half] = G[n, chunk s*2+nh, half*64+d] * t
                    nc.vector.tensor_tensor(
                        _sap(wt, 0, [[32, 64], [2, 16], [1, 2]]),
                        _sap(g, nh * 128, [[1, 64], [256, 16], [64, 2]]),
                        _sap(t_tiles[nh], h * 32, [[0, 64], [2, 16], [1, 2]]),
                        op=ALU.mult)
                    nc.vector.tensor_reduce(
                        _sap(z, nh * 1024 + h * 64, [[1, 64]]),
                        _sap(wt, 0, [[32, 64], [1, 32]]),
                        axis=AX.X, op=ALU.add)

            # ---------- stage G: z -> zT, out projection ----------
            zT = []
            for k in range(KT):
                t = persist.tile([128, N], F32, tag=f"zT_{k}")
                zT.append(t)
            for nh in range(2):
                for c in range(8):
                    pt = tps.tile([128, 128], F32, tag="trps")
                    nc.tensor.transpose(
                        pt[:], z[:, nh * 1024 + c * 128: nh * 1024 + (c + 1) * 128],
                        ident[:])
                    nc.vector.tensor_copy(zT[c][:, nh * 128:(nh + 1) * 128], pt[:])

            for m in range(KT):
                ps = mps.tile([128, N], F32, tag="mlp_ps")
                for k in range(KT):
                    wt = ws.tile([128, 128], F32, tag="wst")
                    nc.sync.dma_start(
                        wt[:], W_out[k * 128:(k + 1) * 128, m * 128:(m + 1) * 128])
                    nc.tensor.matmul(ps[:], wt[:], zT[k][:],
                                     start=(k == 0), stop=(k == KT - 1))
                osb = scr.tile([128, N], F32, tag="osb")
                nc.vector.tensor_scalar(osb[:], ps[:], bo[:, m:m + 1], None,
                                        op0=ALU.add)
                nc.sync.dma_start(outT[m * 128:(m + 1) * 128, :], osb[:])

            # ---------- explicit DRAM deps (gather after v/idx writes) ----------
            from concourse.tile_rust import add_dep_helper
            for gi in gather_insts:
                for w in vwrite_insts:
                    add_dep_helper(gi.ins, w.ins, reason="gather after v write")
            for ld in idxload_insts:
                for w in cwrite_insts:
                    add_dep_helper(ld.ins, w.ins, reason="idx load after idxC write")

    nc.compile()
    return nc


def _prep_inputs(inputs):
    x = np.ascontiguousarray(np.asarray(inputs["x"], dtype=np.float32))
    context = np.asarray(inputs["context"], dtype=np.float32)
    Wv = np.asarray(inputs["Wv"], dtype=np.float32)
    W_off2 = np.asarray(inputs["W_off2"], dtype=np.float32)
    b_off2 = np.asarray(inputs["b_off2"], dtype=np.float32)

    def split(a):
        hi = a.astype(np.bfloat16) if hasattr(np, "bfloat16") else None
        import ml_dtypes
        hi = a.astype(ml_dtypes.bfloat16)
        lo = (a - hi.astype(np.float32)).astype(ml_dtypes.bfloat16)
        return hi, lo

    Wv_hi, Wv_lo = split(Wv)

    cy, h, p = np.meshgrid(np.arange(2), np.arange(HEADS), np.arange(P),
                           indexing="ij")
    oldcol = (h * 16 + p * 2 + cy).reshape(-1)
    W_off2p = np.ascontiguousarray(W_off2[:, oldcol])
    b_off2p = np.ascontiguousarray(b_off2[oldcol])

    common = {
        "Wv_hi": Wv_hi, "Wv_lo": Wv_lo,
        "W_off1": np.ascontiguousarray(np.asarray(inputs["W_off1"], np.float32)),
        "b_off1": np.asarray(inputs["b_off1"], np.float32),
        "W_off2p": W_off2p, "b_off2p": b_off2p,
        "W_att1": np.ascontiguousarray(np.asarray(inputs["W_att1"], np.float32)),
        "b_att1": np.asarray(inputs["b_att1"], np.float32),
        "W_att2": np.ascontiguousarray(np.asarray(inputs["W_att2"], np.float32)),
        "b_att2": np.asarray(inputs["b_att2"], np.float32),
        "W_out": np.ascontiguousarray(np.asarray(inputs["W_out"], np.float32)),
        "b_out": np.asarray(inputs["b_out"], np.float32),
    }

    xflat = x.reshape(B * N, DIM)
    n_idx = np.arange(N)
    in_maps = []
    for b in range(B):
        perm = (n_idx // 32) * 256 + (n_idx % 32) * 8 + b
        xoffT = np.ascontiguousarray(xflat[perm].T)
        xattT = np.ascontiguousarray(x[b].T)
        ctxT = np.ascontiguousarray(context[b].T)
        c_hi, c_lo = split(ctxT)
        m = dict(common)
        m.update({"ctxT_hi": c_hi, "ctxT_lo": c_lo,
                  "xoffT": xoffT, "xattT": xattT})
        in_maps.append(m)
    return in_maps


def kernel(**inputs):
    if "nc" not in _CACHE:
        _CACHE["nc"] = _build()
    nc = _CACHE["nc"]
    in_maps = _prep_inputs(inputs)
    res = run_bass_kernel_spmd(nc, in_maps, list(range(8)))
    _CACHE["last_results"] = res
    out = np.stack([res.results[i]["outT"].T for i in range(B)], axis=0)
    return np.ascontiguousarray(out.astype(np.float32))


# revision 8
# speedup vs baseline: 108.8253x; 1.3076x over previous
"""Trainium2 Bass kernel for nn_DeformableCrossAttention.

Sharding: data-parallel over batch B=8 across 8 NeuronCores (one sample per
core).  Inside each core:
  - offset MLP + attention MLP in fp32 (sampling-position precision matters:
    output error ~ 1.4x the position error in pixels)
  - v = context @ Wv as 3-pass bf16 hi/lo split (hh+hl+lh, ~5e-6 rel err,
    3x faster than native fp32 matmul which runs at 4 cycles/row)
  - bilinear sampling via per-head SWDGE dma_gather of 512B chunks (two
    adjacent-x spatial positions x 64 dh floats) from a per-head
    [spatial, dh] f32 DRAM layout of v
  - attention-weighted bilinear reduce on DVE (elementwise mult with
    broadcast weights + segmented add-reduce)
  - out-projection in fp32, emitted transposed; host transposes back.

Self-contained: hardcodes all shapes from the problem spec.
"""
import sys
sys.path.insert(0, "/opt/trn_rl_repo")

import numpy as np
import concourse.bass as bass
import concourse.mybir as mybir
import concourse.tile as tile
from concourse import bacc
from concourse.bass_utils import run_bass_kernel_spmd
from concourse.masks import make_identity

F32 = mybir.dt.float32
BF16 = mybir.dt.bfloat16
I16 = mybir.dt.int16
I32 = mybir.dt.int32
AF = mybir.ActivationFunctionType
ALU = mybir.AluOpType
AX = mybir.AxisListType

B, N, DIM = 8, 256, 1024
HEADS, DH, P = 16, 64, 8
HS = WS = 64
CTX = HS * WS            # 4096
INNER = HEADS * DH       # 1024
KT = DIM // 128          # 8 k-tiles
PLANE = CTX * DH         # per-head v plane elements (262144)

V_PASSES = int(__import__('os').environ.get('V_PASSES', '3'))  # 1: bf16, 3: bf16 hi/lo split (hh + hl + lh)
CTX_SUP = 512            # ctx supertile rows
N_SUP = CTX // CTX_SUP   # 8 supertiles
M_PER_SUP = CTX_SUP // 128

_CACHE = {}


def _ap(t, offset, dims):
    return bass.AP(t.ap().tensor if hasattr(t, "ap") else t.tensor, offset, dims)


def _sap(tile_obj, extra, dims):
    """Sub-AP of an SBUF tile: keep its partition dim, custom free dims,
    extra offset in elements."""
    a = tile_obj[:]
    return bass.AP(a.tensor, a.offset + extra, [list(a.ap[0])] + dims)


def _build(repeat=1):
    nc = bacc.Bacc("TRN2", target_bir_lowering=False, debug=False)

    # ---------------- I/O ----------------
    ctxT_hi = nc.dram_tensor("ctxT_hi", [DIM, CTX], BF16, kind="ExternalInput")
    ctxT_lo = nc.dram_tensor("ctxT_lo", [DIM, CTX], BF16, kind="ExternalInput")
    Wv_hi = nc.dram_tensor("Wv_hi", [DIM, INNER], BF16, kind="ExternalInput")
    Wv_lo = nc.dram_tensor("Wv_lo", [DIM, INNER], BF16, kind="ExternalInput")
    xoffT = nc.dram_tensor("xoffT", [DIM, N], F32, kind="ExternalInput")
    xattT = nc.dram_tensor("xattT", [DIM, N], F32, kind="ExternalInput")
    W_off1 = nc.dram_tensor("W_off1", [DIM, DIM], F32, kind="ExternalInput")
    b_off1 = nc.dram_tensor("b_off1", [DIM], F32, kind="ExternalInput")
    W_off2p = nc.dram_tensor("W_off2p", [DIM, 256], F32, kind="ExternalInput")
    b_off2p = nc.dram_tensor("b_off2p", [256], F32, kind="ExternalInput")
    W_att1 = nc.dram_tensor("W_att1", [DIM, DIM], F32, kind="ExternalInput")
    b_att1 = nc.dram_tensor("b_att1", [DIM], F32, kind="ExternalInput")
    W_att2 = nc.dram_tensor("W_att2", [DIM, 128], F32, kind="ExternalInput")
    b_att2 = nc.dram_tensor("b_att2", [128], F32, kind="ExternalInput")
    W_out = nc.dram_tensor("W_out", [INNER, DIM], F32, kind="ExternalInput")
    b_out = nc.dram_tensor("b_out", [DIM], F32, kind="ExternalInput")

    outT = nc.dram_tensor("outT", [DIM, N], F32, kind="ExternalOutput")

    # DRAM scratch
    v_dram = nc.dram_tensor("v_dram", [HEADS, PLANE], F32)
    # idxC[q, h, cy, p, m] int16
    idxC = nc.dram_tensor("idxC", [16, HEADS, 2, P, 16], I16)

    vwrite_insts = []
    cwrite_insts = []
    gather_insts = []
    idxload_insts = []

    with tile.TileContext(nc) as tc:
        import contextlib
        with contextlib.ExitStack() as ctx:
            persist = ctx.enter_context(tc.tile_pool(name="persist", bufs=1))
            ws = ctx.enter_context(tc.tile_pool(name="wstream", bufs=6))
            h1p = ctx.enter_context(tc.tile_pool(name="h1p", bufs=1))
            ctxp = ctx.enter_context(tc.tile_pool(name="ctxp", bufs=1))
            vsbp = ctx.enter_context(tc.tile_pool(name="vsbp", bufs=2))
            gp = ctx.enter_context(tc.tile_pool(name="gp", bufs=2))
            wtp = ctx.enter_context(tc.tile_pool(name="wtp", bufs=1))
            scr = ctx.enter_context(tc.tile_pool(name="scr", bufs=1))
            mps = ctx.enter_context(tc.tile_pool(name="mps", bufs=2, space="PSUM"))
            vps = ctx.enter_context(tc.tile_pool(name="vps", bufs=2, space="PSUM"))
            tps = ctx.enter_context(tc.tile_pool(name="tps", bufs=2, space="PSUM"))
            if repeat > 1:
                ctx.enter_context(tc.For_i(0, repeat, 1))

            # ---------- persistent loads ----------
            def load_tiles(dram, rows, cols, dt, tag, ncols=None):
                ncols = cols if ncols is None else ncols
                ts_ = []
                for k in range(rows // 128):
                    t = persist.tile([128, ncols], dt, tag=f"{tag}_{k}")
                    nc.sync.dma_start(t[:], dram[k * 128:(k + 1) * 128, :])
                    ts_.append(t)
                return ts_

            wv_hi = load_tiles(Wv_hi, DIM, INNER, BF16, "wvh")
            wv_lo = load_tiles(Wv_lo, DIM, INNER, BF16, "wvl") if V_PASSES == 3 else None
            woff2 = load_tiles(W_off2p, DIM, 256, F32, "wo2")
            watt2 = load_tiles(W_att2, DIM, 128, F32, "wa2")

            def load_bias(dram, n_elem, tag):
                k = n_elem // 128
                t = persist.tile([128, k], F32, tag=tag)
                nc.sync.dma_start(t[:], _ap(dram, 0, [[1, 128], [128, k]]))
                return t

            bo1 = load_bias(b_off1, DIM, "bo1")
            bo2 = load_bias(b_off2p, 256, "bo2")
            ba1 = load_bias(b_att1, DIM, "ba1")
            ba2 = load_bias(b_att2, 128, "ba2")
            bo = load_bias(b_out, DIM, "bo")

            ident = persist.tile([128, 128], F32, tag="ident")
            make_identity(nc, ident[:])

            # ---------- MLP helper: yT[m] = act(sum_k W[k,m]^T @ xT[k] + b[m]) ----------
            def mlp_layer(w_dram, x_tiles, bias_tile, mtiles, act, out_tag, pool):
                outs = []
                for m in range(mtiles):
                    ps = mps.tile([128, N], F32, tag="mlp_ps")
                    for k in range(KT):
                        wt = ws.tile([128, 128], F32, tag="wst")
                        nc.sync.dma_start(
                            wt[:], w_dram[k * 128:(k + 1) * 128,
                                          m * 128:(m + 1) * 128])
                        nc.tensor.matmul(ps[:], wt[:], x_tiles[k][:],
                                         start=(k == 0), stop=(k == KT - 1))
                    o = pool.tile([128, N], F32, tag=f"{out_tag}_{m}")
                    nc.scalar.activation(o[:], ps[:], act,
                                         bias=bias_tile[:, m:m + 1])
                    outs.append(o)
                return outs

            def mlp_layer2(w_tiles, x_tiles, bias_tile, mtiles, act, out_tag,
                           use_dve_bias=False):
                outs = []
                for m in range(mtiles):
                    ps = mps.tile([128, N], F32, tag="mlp_ps")
                    for k in range(KT):
                        nc.tensor.matmul(ps[:], w_tiles[k][:, m * 128:(m + 1) * 128],
                                         x_tiles[k][:],
                                         start=(k == 0), stop=(k == KT - 1))
                    o = scr.tile([128, N], F32, tag=f"{out_tag}_{m}")
                    if use_dve_bias:
                        nc.vector.tensor_scalar(o[:], ps[:], bias_tile[:, m:m + 1],
                                                None, op0=ALU.add)
                    else:
                        nc.scalar.activation(o[:], ps[:], act,
                                             bias=bias_tile[:, m:m + 1])
                    outs.append(o)
                return outs

            # ---------- stage A: offset MLP (fp32) ----------
            xoff_t = []
            for k in range(KT):
                t = h1p.tile([128, N], F32, tag=f"xt_{k}")
                nc.sync.dma_start(t[:], xoffT[k * 128:(k + 1) * 128, :])
                xoff_t.append(t)
            h1 = mlp_layer(W_off1, xoff_t, bo1, KT, AF.Gelu, "h1", h1p)
            # off2: 2 m-tiles -> lxT (cy=0), lyT (cy=1), tanh applied
            loff = mlp_layer2(woff2, h1, bo2, 2, AF.Tanh, "loff")
            lxT, lyT = loff

            # ---------- stage B: attention MLP (fp32) ----------
            xatt_t = []
            for k in range(KT):
                t = h1p.tile([128, N], F32, tag=f"xt_{k}")
                nc.sync.dma_start(t[:], xattT[k * 128:(k + 1) * 128, :])
                xatt_t.append(t)
            g1 = mlp_layer(W_att1, xatt_t, ba1, KT, AF.Gelu, "h1", h1p)
            attT = mlp_layer2(watt2, g1, ba2, 1, AF.Copy, "attT",
                              use_dve_bias=True)[0]

            # ---------- stage C: PE transposes to [n, hp] ----------
            def transpose_128x256(src, tag):
                halves = []
                for i in range(2):
                    pt = tps.tile([128, 128], F32, tag="trps")
                    nc.tensor.transpose(pt[:], src[:, i * 128:(i + 1) * 128],
                                        ident[:])
                    o = scr.tile([128, 128], F32, tag=f"{tag}_{i}")
                    nc.vector.tensor_copy(o[:], pt[:])
                    halves.append(o)
                return halves

            lx_n = transpose_128x256(lxT, "lxn")   # [n-tile][128, 128hp]
            ly_n = transpose_128x256(lyT, "lyn")
            att_n = transpose_128x256(attT, "attn")

            # ---------- stage D1: positions in [hp, n] for gather indices ----------
            # g' = tanh*31.5 + 31.0 ; min 62.4999 ; xi = round(g'), xf float
            def pos_chain_T(lt, tag):
                gp_ = scr.tile([128, N], F32, tag=f"{tag}_g")
                nc.vector.tensor_scalar(gp_[:], lt[:], 31.5, 31.0,
                                        op0=ALU.mult, op1=ALU.add)
                nc.vector.tensor_scalar(gp_[:], gp_[:], 62.4999, None, op0=ALU.min)
                xi = scr.tile([128, N], I32, tag=f"{tag}_i")
                nc.vector.tensor_copy(xi[:], gp_[:])
                xf = scr.tile([128, N], F32, tag=f"{tag}_f")
                nc.vector.tensor_copy(xf[:], xi[:])
                return gp_, xf

            _, xfT = pos_chain_T(lxT, "pxT")
            _, yfT = pos_chain_T(lyT, "pyT")
            idx0f = scr.tile([128, N], F32, tag="idx0f")
            nc.vector.scalar_tensor_tensor(idx0f[:], yfT[:], 64.0, xfT[:],
                                           op0=ALU.mult, op1=ALU.add)
            idx1f = scr.tile([128, N], F32, tag="idx1f")
            nc.vector.tensor_scalar(idx1f[:], idx0f[:], 64.0, None, op0=ALU.add)

            for cy, srcf in enumerate([idx0f, idx1f]):
                ii = scr.tile([128, N], I16, tag=f"idxi_{cy}")
                nc.vector.tensor_copy(ii[:], srcf[:])
                # free transpose: Sg[hp, q*16+m] = ii[hp, m*16+q]
                sg = scr.tile([128, N], I16, tag=f"sg_{cy}")
                nc.vector.tensor_copy(
                    sg[:], _sap(ii, 0, [[1, 16], [16, 16]]))
                # write to idxC[q, h, cy, p, m]: one DMA per q (3-dim AP cap)
                for q in range(16):
                    dst = bass.AP(idxC.ap().tensor, q * 4096 + cy * 128,
                                  [[256, 16], [16, 8], [1, 16]])
                    w = nc.sync.dma_start(dst, sg[:, q * 16:(q + 1) * 16])
                    cwrite_insts.append(w)

            # ---------- stage D2: lerp weights + softmax in [n, hp] ----------
            # t[n, col = h*32 + cy*16 + p*2 + half]
            t_tiles = []
            for nh in range(2):
                gx_ = scr.tile([128, 128], F32, tag=f"gxn_{nh}")
                nc.vector.tensor_scalar(gx_[:], lx_n[nh][:], 31.5, 31.0,
                                        op0=ALU.mult, op1=ALU.add)
                nc.vector.tensor_scalar(gx_[:], gx_[:], 62.4999, None, op0=ALU.min)
                xi_ = scr.tile([128, 128], I32, tag=f"xin_{nh}")
                nc.vector.tensor_copy(xi_[:], gx_[:])
                xf_ = scr.tile([128, 128], F32, tag=f"xfn_{nh}")
                nc.vector.tensor_copy(xf_[:], xi_[:])
                wx = scr.tile([128, 128], F32, tag=f"wx_{nh}")
                # wx = (g' + 0.5) - xf
                nc.vector.scalar_tensor_tensor(wx[:], gx_[:], 0.5, xf_[:],
                                               op0=ALU.add, op1=ALU.subtract)
                gy_ = scr.tile([128, 128], F32, tag=f"gyn_{nh}")
                nc.vector.tensor_scalar(gy_[:], ly_n[nh][:], 31.5, 31.0,
                                        op0=ALU.mult, op1=ALU.add)
                nc.vector.tensor_scalar(gy_[:], gy_[:], 62.4999, None, op0=ALU.min)
                yi_ = scr.tile([128, 128], I32, tag=f"yin_{nh}")
                nc.vector.tensor_copy(yi_[:], gy_[:])
                yf_ = scr.tile([128, 128], F32, tag=f"yfn_{nh}")
                nc.vector.tensor_copy(yf_[:], yi_[:])
                wy = scr.tile([128, 128], F32, tag=f"wy_{nh}")
                nc.vector.scalar_tensor_tensor(wy[:], gy_[:], 0.5, yf_[:],
                                               op0=ALU.add, op1=ALU.subtract)

                # softmax over p (groups of 8 along free)
                an = att_n[nh]
                mx = scr.tile([128, 16], F32, tag=f"mx_{nh}")
                nc.vector.tensor_reduce(
                    mx[:], _sap(an, 0, [[8, 16], [1, 8]]),
                    axis=AX.X, op=ALU.max)
                ex = scr.tile([128, 128], F32, tag=f"ex_{nh}")
                nc.vector.tensor_tensor(
                    _sap(ex, 0, [[8, 16], [1, 8]]),
                    _sap(an, 0, [[8, 16], [1, 8]]),
                    _sap(mx, 0, [[1, 16], [0, 8]]),
                    op=ALU.subtract)
                nc.scalar.activation(ex[:], ex[:], AF.Exp)
                sm = scr.tile([128, 16], F32, tag=f"sm_{nh}")
                nc.vector.tensor_reduce(
                    sm[:], _sap(ex, 0, [[8, 16], [1, 8]]),
                    axis=AX.X, op=ALU.add)
                rs = scr.tile([128, 16], F32, tag=f"rs_{nh}")
                nc.vector.reciprocal(rs[:], sm[:])
                aw = scr.tile([128, 128], F32, tag=f"aw_{nh}")
                nc.vector.tensor_tensor(
                    _sap(aw, 0, [[8, 16], [1, 8]]),
                    _sap(ex, 0, [[8, 16], [1, 8]]),
                    _sap(rs, 0, [[1, 16], [0, 8]]),
                    op=ALU.mult)

                # u0 = aw*(1-wx) = aw - aw*wx ; u1 = aw*wx
                u1 = scr.tile([128, 128], F32, tag=f"u1_{nh}")
                nc.vector.tensor_tensor(u1[:], aw[:], wx[:], op=ALU.mult)
                u0 = scr.tile([128, 128], F32, tag=f"u0_{nh}")
                nc.vector.tensor_tensor(u0[:], aw[:], u1[:], op=ALU.subtract)
                cw1 = wy
                cw0 = scr.tile([128, 128], F32, tag=f"cw0_{nh}")
                nc.vector.tensor_scalar(cw0[:], wy[:], -1.0, 1.0,
                                        op0=ALU.mult, op1=ALU.add)

                tt = scr.tile([128, 512], F32, tag=f"tt_{nh}")
                for cyv, cw in ((0, cw0), (1, cw1)):
                    for half, u in ((0, u0), (1, u1)):
                        nc.vector.tensor_tensor(
                            _sap(tt, cyv * 16 + half, [[32, 16], [2, 8]]),
                            _sap(u, 0, [[8, 16], [1, 8]]),
                            _sap(cw, 0, [[8, 16], [1, 8]]),
                            op=ALU.mult)
                t_tiles.append(tt)

            # ---------- stage E: v matmul + store ----------
            for sup in range(N_SUP):
                c0 = sup * CTX_SUP
                chi, clo = [], []
                for k in range(KT):
                    t = ctxp.tile([128, CTX_SUP], BF16, tag=f"ch_{k}")
                    nc.sync.dma_start(t[:], ctxT_hi[k * 128:(k + 1) * 128,
                                                    c0:c0 + CTX_SUP])
                    chi.append(t)
                    if V_PASSES == 3:
                        t2 = ctxp.tile([128, CTX_SUP], BF16, tag=f"cl_{k}")
                        nc.sync.dma_start(t2[:], ctxT_lo[k * 128:(k + 1) * 128,
                                                         c0:c0 + CTX_SUP])
                        clo.append(t2)
                for mm in range(M_PER_SUP):
                    msl = slice(mm * 128, (mm + 1) * 128)
                    pss = [vps.tile([128, 512], F32, tag=f"vps_{h2}",
                                    name=f"vps_{sup}_{mm}_{h2}")
                           for h2 in range(2)]
                    passes = [(chi, wv_hi)]
                    if V_PASSES == 3:
                        passes += [(chi, wv_lo), (clo, wv_hi)]
                    np_ = len(passes)
                    for pi, (lhs_t, rhs_t) in enumerate(passes):
                        for k in range(KT):
                            first = (pi == 0 and k == 0)
                            last = (pi == np_ - 1 and k == KT - 1)
                            for h2 in range(2):
                                nc.tensor.matmul(
                                    pss[h2][:], lhs_t[k][:, msl],
                                    rhs_t[k][:, h2 * 512:(h2 + 1) * 512],
                                    start=first, stop=last)
                    vsb = vsbp.tile([128, 1024], F32, tag="vsb")
                    for h2 in range(2):
                        nc.scalar.copy(vsb[:, h2 * 512:(h2 + 1) * 512], pss[h2][:])
                    # store: v_dram[h, (c0+mm*128 + r)*64 + d]
                    dst = bass.AP(v_dram.ap().tensor, (c0 + mm * 128) * DH,
                                  [[DH, 128], [PLANE, HEADS], [1, DH]])
                    w = nc.sync.dma_start(dst, vsb[:])
                    vwrite_insts.append(w)

            # ---------- stage F: per-head gather + reduce ----------
            z = persist.tile([128, 2048], F32, tag="z")  # col = nh*1024 + h*64 + d
            # all-heads idx tile [128, 4096]: col = h*256 + cy*128 + p*16 + m,
            # partitions = 8 replicas x 16 q
            all_idx = persist.tile([128, 4096], I16, tag="all_idx")
            for gi_ in range(8):
                src = bass.AP(idxC.ap().tensor, 0, [[4096, 16], [1, 4096]])
                ld = nc.sync.dma_start(all_idx[gi_ * 16:(gi_ + 1) * 16, :], src)
                idxload_insts.append(ld)

            for h in range(HEADS):
                g = gp.tile([128, 32, 128], F32, tag="g")
                vsrc = bass.AP(v_dram.ap().tensor, h * PLANE,
                               [[DH, CTX - 1], [1, 2 * DH]])
                gi = nc.gpsimd.dma_gather(g[:], vsrc,
                                          all_idx[:, h * 256:(h + 1) * 256],
                                          4096, 4096,
                                          2 * DH, elem_step=DH,
                                          single_packet=False)
                gather_insts.append(gi)

                for nh in range(2):
                    wt = wtp.tile([128, 2048], F32, tag="wt")
                    # wt[d*32 + s*2 + half] = G[n, chunk s*2+nh, half*64+d] * t
                    nc.vector.tensor_tensor(
                        _sap(wt, 0, [[32, 64], [2, 16], [1, 2]]),
                        _sap(g, nh * 128, [[1, 64], [256, 16], [64, 2]]),
                        _sap(t_tiles[nh], h * 32, [[0, 64], [2, 16], [1, 2]]),
                        op=ALU.mult)
                    nc.vector.tensor_reduce(
                        _sap(z, nh * 1024 + h * 64, [[1, 64]]),
                        _sap(wt, 0, [[32, 64], [1, 32]]),
                        axis=AX.X, op=ALU.add)

            # ---------- stage G: z -> zT, out projection ----------
            zT = []
            for k in range(KT):
                t = persist.tile([128, N], F32, tag=f"zT_{k}")
                zT.append(t)
            for nh in range(2):
                for c in range(8):
                    pt = tps.tile([128, 128], F32, tag="trps")
                    nc.tensor.transpose(
                        pt[:], z[:, nh * 1024 + c * 128: nh * 1024 + (c + 1) * 128],
                        ident[:])
                    nc.vector.tensor_copy(zT[c][:, nh * 128:(nh + 1) * 128], pt[:])

            for m in range(KT):
                ps = mps.tile([128, N], F32, tag="mlp_ps")
                for k in range(KT):
                    wt = ws.tile([128, 128], F32, tag="wst")
                    nc.sync.dma_start(
                        wt[:], W_out[k * 128:(k + 1) * 128, m * 128:(m + 1) * 128])
                    nc.tensor.matmul(ps[:], wt[:], zT[k][:],
                                     start=(k == 0), stop=(k == KT - 1))
                osb = scr.tile([128, N], F32, tag="osb")
                nc.vector.tensor_scalar(osb[:], ps[:], bo[:, m:m + 1], None,
                                        op0=ALU.add)
                nc.sync.dma_start(outT[m * 128:(m + 1) * 128, :], osb[:])

            # ---------- explicit DRAM deps (gather after v/idx writes) ----------
            from concourse.tile_rust import add_dep_helper
            for gi in gather_insts:
                for w in vwrite_insts:
                    add_dep_helper(gi.ins, w.ins, reason="gather after v write")
            for ld in idxload_insts:
                for w in cwrite_insts:
                    add_dep_helper(ld.ins, w.ins, reason="idx load after idxC write")

    nc.compile()
    return nc


def _prep_inputs(inputs):
    x = np.ascontiguousarray(np.asarray(inputs["x"], dtype=np.float32))
    context = np.asarray(inputs["context"], dtype=np.float32)
    Wv = np.asarray(inputs["Wv"], dtype=np.float32)
    W_off2 = np.asarray(inputs["W_off2"], dtype=np.float32)
    b_off2 = np.asarray(inputs["b_off2"], dtype=np.float32)

    def split(a):
        hi = a.astype(np.bfloat16) if hasattr(np, "bfloat16") else None
        import ml_dtypes
        hi = a.astype(ml_dtypes.bfloat16)
        lo = (a - hi.astype(np.float32)).astype(ml_dtypes.bfloat16)
        return hi, lo

    Wv_hi, Wv_lo = split(Wv)

    cy, h, p = np.meshgrid(np.arange(2), np.arange(HEADS), np.arange(P),
                           indexing="ij")
    oldcol = (h * 16 + p * 2 + cy).reshape(-1)
    W_off2p = np.ascontiguousarray(W_off2[:, oldcol])
    b_off2p = np.ascontiguousarray(b_off2[oldcol])

    common = {
        "Wv_hi": Wv_hi, "Wv_lo": Wv_lo,
        "W_off1": np.ascontiguousarray(np.asarray(inputs["W_off1"], np.float32)),
        "b_off1": np.asarray(inputs["b_off1"], np.float32),
        "W_off2p": W_off2p, "b_off2p": b_off2p,
        "W_att1": np.ascontiguousarray(np.asarray(inputs["W_att1"], np.float32)),
        "b_att1": np.asarray(inputs["b_att1"], np.float32),
        "W_att2": np.ascontiguousarray(np.asarray(inputs["W_att2"], np.float32)),
        "b_att2": np.asarray(inputs["b_att2"], np.float32),
        "W_out": np.ascontiguousarray(np.asarray(inputs["W_out"], np.float32)),
        "b_out": np.asarray(inputs["b_out"], np.float32),
    }

    xflat = x.reshape(B * N, DIM)
    n_idx = np.arange(N)
    in_maps = []
    for b in range(B):
        perm = (n_idx // 32) * 256 + (n_idx % 32) * 8 + b
        xoffT = np.ascontiguousarray(xflat[perm].T)
        xattT = np.ascontiguousarray(x[b].T)
        ctxT = np.ascontiguousarray(context[b].T)
        c_hi, c_lo = split(ctxT)
        m = dict(common)
        m.update({"ctxT_hi": c_hi, "ctxT_lo": c_lo,
                  "xoffT": xoffT, "xattT": xattT})
        in_maps.append(m)
    return in_maps


def kernel(**inputs):
    if "nc" not in _CACHE:
        _CACHE["nc"] = _build()
    nc = _CACHE["nc"]
    in_maps = _prep_inputs(inputs)
    res = run_bass_kernel_spmd(nc, in_maps, list(range(8)))
    _CACHE["last_results"] = res
    out = np.stack([res.results[i]["outT"].T for i in range(B)], axis=0)
    return np.ascontiguousarray(out.astype(np.float32))
